# revision 8
# baseline (speedup 1.0000x reference)
"""AdaptiveTripletLoss on 8 TRN2 NeuronCores.

Device: the compute-dominant Gram matrix G = E @ E^T in fp8 DoubleRow on
the PE, symmetry-aware (upper-triangular blocks only). Generic chain
machinery: each core loads NSLOT packed 512-column groups (each a pair
of 256-row half-groups, 4 chunks of 4 k-tiles per slot for fine-grained
DMA/compute overlap) and runs a fixed shared CHAINS schedule; the host
picks per-core slot contents so the union covers all of upper(G).
Dummy warm-up matmuls un-throttle the PE clock (HAM) while input DMAs
stream. Host mirrors blocks, then does masks/counts, order-statistic
selection, exact d_ap/d_an norms and the masked mean.
"""

import os

import numpy as np
import ml_dtypes

N, D = 4096, 2048
NUM_IDS = 512
N_CORES = 8
MARGIN = 0.3
RATIOS = (0.3, 0.4, 0.3)
EPS = 1e-6

B = 512           # block edge / slot width
HALF = 256        # half-group rows
KT = D // 128     # 16 k-tiles per slot
NCHUNK = 4        # 4 k-tiles per chunk
TT = KT // 2      # 8 DoubleRow steps per chain
N_WARM = 5        # dummy warm-up matmuls bridging memset -> first data

LAST_EXEC_NS = None

# ---- cover definition (v2: 5 full-group slots, 20 chains) ----
_GROUP_SLOTS = [
    (0, 1, 2, 3, 4), (0, 1, 4, 5, 6), (2, 3, 6, 7, 5), (2, 3, 4, 5, 7),
    (4, 5, 6, 7, 0), (0, 1, 6, 7, 2), (0, 4, 1, 5, 1), (2, 6, 3, 7, 3),
]
_CELLS = ((0, 2), (1, 2), (0, 3), (1, 3), (4, 4))

NSLOT = 5
# SLOTPACK[core][slot] = (half-group, half-group): rows h*256..h*256+255
SLOTPACK = [[(2 * g, 2 * g + 1) for g in gs] for gs in _GROUP_SLOTS]
# shared schedule: chain = (lhs_slot, m, rhs_slot); grouped per 4 for
# psum interleaving
CHAINS = [(ls, m, rs) for (ls, rs) in _CELLS for m in range(4)]
CHAIN_GROUPS = [list(range(i, min(i + 4, len(CHAINS))))
                for i in range(0, len(CHAINS), 4)]
NCHAIN = len(CHAINS)


def _dma_order():
    """Input chunk order: first chain-group's slots chunk-interleaved,
    then remaining slots in first-use order."""
    first = []
    for ci in CHAIN_GROUPS[0]:
        ls, _, rs = CHAINS[ci]
        for s in (ls, rs):
            if s not in first:
                first.append(s)
    rest = []
    for (ls, _, rs) in CHAINS:
        for s in (ls, rs):
            if s not in first and s not in rest:
                rest.append(s)
    order = []
    for c in range(NCHUNK):
        for s in first:
            order.append((s, c))
    for s in rest:
        for c in range(NCHUNK):
            order.append((s, c))
    return order


def _build_gram_kernel():
    import concourse.bacc as bacc
    import concourse.tile as tile
    from concourse import mybir

    nc = bacc.Bacc(None, target_bir_lowering=False)

    f32 = mybir.dt.float32
    bf16 = mybir.dt.bfloat16
    fp8 = mybir.dt.float8e4

    grps = nc.declare_dram_parameter("grps", [NSLOT, 128, KT, B], fp8,
                                     isOutput=False)
    out = nc.declare_dram_parameter("out", [NCHAIN, 128, B], bf16,
                                    isOutput=True)

    with tile.TileContext(nc) as tc:
        with (
            tc.tile_pool(name="grp_p", bufs=1) as grp_pool,
            tc.tile_pool(name="psum", bufs=8, space="PSUM") as psum_pool,
            tc.tile_pool(name="outp", bufs=6) as out_pool,
        ):
            gch = [[grp_pool.tile([128, NCHUNK, B], fp8, name=f"g{s}_{c}")
                    for c in range(NCHUNK)] for s in range(NSLOT)]
            dmy = grp_pool.tile([128, 2, B], fp8, name="dmy")

            for s, c in _dma_order():
                k0 = c * NCHUNK
                nc.sync.dma_start(gch[s][c][:], grps[s, :, k0:k0 + NCHUNK, :])

            # PE warm-up on a zeroed tile while inputs stream.
            nc.vector.memset(dmy[:], 0.0)
            for i in range(N_WARM):
                wp = psum_pool.tile([128, B], f32, name="ps")
                nc.tensor.matmul(
                    wp[:], dmy[:, :, 0:128], dmy[:],
                    start=True, stop=True,
                    perf_mode=mybir.MatmulPerfMode.DoubleRow,
                )

            for grp in CHAIN_GROUPS:
                pss = [psum_pool.tile([128, B], f32, name="ps") for _ in grp]
                for t in range(TT):
                    ct = t // 2
                    o = 2 * (t % 2)
                    for j, ci in enumerate(grp):
                        ls, m, rs = CHAINS[ci]
                        nc.tensor.matmul(
                            pss[j][:],
                            gch[ls][ct][:, o:o + 2, m * 128:(m + 1) * 128],
                            gch[rs][ct][:, o:o + 2, :],
                            start=(t == 0),
                            stop=(t == TT - 1),
                            perf_mode=mybir.MatmulPerfMode.DoubleRow,
                        )
                for j, ci in enumerate(grp):
                    ot = out_pool.tile([128, B], bf16, name="ot")
                    nc.vector.tensor_copy(ot[:], pss[j][:])
                    nc.scalar.dma_start(out[ci], ot[:])

    nc.compile()
    return nc


_NC_CACHE = None


def _pack_slot(eT8: np.ndarray, pair) -> np.ndarray:
    """eT8 [D, N] fp8 -> [128, KT, B] packed slot of two half-groups."""
    h0, h1 = pair
    blk = np.concatenate(
        [eT8[:, h0 * HALF:(h0 + 1) * HALF], eT8[:, h1 * HALF:(h1 + 1) * HALF]],
        axis=1)                                      # [2048, 512]
    return np.ascontiguousarray(
        blk.reshape(KT, 128, B).transpose(1, 0, 2))  # [128, 16, 512]


def _run_gram(emb: np.ndarray) -> np.ndarray:
    """Run the 8-core symmetric Gram kernel; returns G = emb @ emb.T f32."""
    global _NC_CACHE, LAST_EXEC_NS
    from concourse.bass_utils import run_bass_kernel_spmd

    if _NC_CACHE is None:
        _NC_CACHE = _build_gram_kernel()
    nc = _NC_CACHE

    eT8 = np.ascontiguousarray(emb.T).astype(ml_dtypes.float8_e4m3)
    pack_cache = {}
    in_maps = []
    for core in range(N_CORES):
        slabs = []
        for pair in SLOTPACK[core]:
            if pair not in pack_cache:
                pack_cache[pair] = _pack_slot(eT8, pair)
            slabs.append(pack_cache[pair])
        in_maps.append({"grps": np.ascontiguousarray(np.stack(slabs, axis=0))})

    trace = bool(int(os.environ.get("KERNEL_TRACE", "0")))
    res = run_bass_kernel_spmd(
        nc, in_maps, core_ids=list(range(N_CORES)), trace=trace
    )
    if res.exec_time_ns is not None:
        LAST_EXEC_NS = res.exec_time_ns

    G = np.empty((N, N), dtype=np.float32)
    for core in range(N_CORES):
        o = np.asarray(res.results[core]["out"], dtype=np.float32)  # [NCHAIN,128,B]
        S = SLOTPACK[core]
        for ci, (ls, m, rs) in enumerate(CHAINS):
            r0 = S[ls][m // 2] * HALF + (m % 2) * 128
            strip = o[ci]                       # [128, 512]
            for half in range(2):
                c0 = S[rs][half] * HALF
                piece = strip[:, half * HALF:(half + 1) * HALF]  # [128, 256]
                G[r0:r0 + 128, c0:c0 + HALF] = piece
                G[c0:c0 + HALF, r0:r0 + 128] = piece.T
    return G


def _sample_js(counts: np.ndarray, us: list) -> np.ndarray:
    """Replicate the reference's f32 sampling math. counts [N] int, us 3x[N]
    f32 uniforms. Returns j ranks [N, 3] int64 (rank into the masked sort)."""
    out = []
    for t, r in enumerate(RATIOS):
        cnt = np.maximum(
            np.int32(1),
            np.floor(counts.astype(np.float32) * np.float32(r)).astype(np.int32),
        )
        j = np.minimum((us[t] * cnt.astype(np.float32)).astype(np.int32), cnt - 1)
        out.append(j.astype(np.int64))
    return np.stack(out, axis=1)


def kernel(embeddings: np.ndarray, labels: np.ndarray) -> np.ndarray:
    emb = np.ascontiguousarray(np.asarray(embeddings, dtype=np.float32))
    lab = np.asarray(labels).astype(np.int64)

    G = _run_gram(emb)

    # Selection keys: within row i, ordering by (sq_j - 2 G[i,j]) equals
    # ordering by distance.
    sq = np.einsum("ij,ij->i", emb, emb).astype(np.float32)

    # Uniforms must match jax.random with key 42 bit-exactly.
    import jax

    with jax.default_device(jax.devices("cpu")[0]):
        skey = jax.random.key(42)
        keys = jax.random.split(skey, 6)
        us = [np.asarray(jax.random.uniform(k, (N,))) for k in keys]

    class_size = np.bincount(lab, minlength=NUM_IDS)
    pos_count = class_size[lab] - 1
    neg_count = N - class_size[lab]
    valid = (pos_count > 0) & (neg_count > 0)

    pos_js = _sample_js(pos_count, us[0:3])  # [N, 3]
    neg_js = _sample_js(neg_count, us[3:6])  # [N, 3]

    # Per-class member lists
    order = np.argsort(lab, kind="stable")
    sorted_lab = lab[order]
    starts = np.searchsorted(sorted_lab, np.arange(NUM_IDS), side="left")
    ends = np.searchsorted(sorted_lab, np.arange(NUM_IDS), side="right")

    pos_idx = np.zeros((N, 3), dtype=np.int64)
    neg_idx = np.zeros((N, 3), dtype=np.int64)
    INF = np.float32(np.inf)

    for i in range(N):
        li = lab[i]
        members = order[starts[li]:ends[li]]
        key_row = sq - 2.0 * G[i]  # f32 [N]
        if valid[i]:
            pos_members = members[members != i]
            pk = key_row[pos_members]
            po = np.argsort(pk, kind="stable")
            pos_idx[i] = pos_members[po[pos_js[i]]]
        # negatives: mask out own class and self
        nk = key_row.copy()
        nk[members] = INF
        nk[i] = INF
        kth = np.unique(neg_js[i])
        part = np.argpartition(nk, kth)
        neg_idx[i] = part[neg_js[i]]

    a = emb[:, None, :]
    p = emb[pos_idx]
    ng = emb[neg_idx]
    d_ap = np.sqrt(np.sum((a - p + np.float32(EPS)) ** 2, axis=-1))
    d_an = np.sqrt(np.sum((a - ng + np.float32(EPS)) ** 2, axis=-1))
    tri = np.maximum(d_ap - d_an + np.float32(MARGIN), np.float32(0.0))
    w = valid[:, None].astype(np.float32)
    denom = max(3.0 * float(valid.sum()), 1.0)
    loss = np.float32(np.sum(tri * w) / denom)
    return np.array(loss, dtype=np.float32)


# revision 10
# speedup vs baseline: 1.0700x; 1.0700x over previous
"""AdaptiveTripletLoss on 8 TRN2 NeuronCores.

Device: the compute-dominant Gram matrix G = E @ E^T in fp8 DoubleRow on
the PE, symmetry-aware (upper-triangular blocks only). Generic chain
machinery: each core loads NSLOT packed 512-column groups (each a pair
of 256-row half-groups, 4 chunks of 4 k-tiles per slot for fine-grained
DMA/compute overlap) and runs a fixed shared CHAINS schedule; the host
picks per-core slot contents so the union covers all of upper(G).
Dummy warm-up matmuls un-throttle the PE clock (HAM) while input DMAs
stream. Host mirrors blocks, then does masks/counts, order-statistic
selection, exact d_ap/d_an norms and the masked mean.
"""

import os

import numpy as np
import ml_dtypes

N, D = 4096, 2048
NUM_IDS = 512
N_CORES = 8
MARGIN = 0.3
RATIOS = (0.3, 0.4, 0.3)
EPS = 1e-6

B = 512           # block edge / slot width
HALF = 256        # half-group rows
KT = D // 128     # 16 k-tiles per slot
NCHUNK = 4        # 4 k-tiles per chunk
TT = KT // 2      # 8 DoubleRow steps per chain

LAST_EXEC_NS = None

# ---- cover definition (T6: 18 chains/core, 144 MMs) ----
# Half-group supers: super s = half-groups (2s, 2s+1). K16 minus the
# matching M equals K8 over supers with every edge blown up to K2,2;
# each core's cross coverage is the path P3-P0-P2-P1 (l-i-k-j) of a
# P4-decomposition of K8-F, the shared F-edge (j,l) is covered half by
# each core of a pair via the ordered P1 slot, and slot 4 is the core's
# own loop super (diag half-blocks).
NSLOT = 5
# SLOTPACK[core][slot] = (half-group, half-group): rows h*256..h*256+255
SLOTPACK = [
    [(4, 5), (2, 3), (8, 9), (0, 1), (6, 7)],
    [(6, 7), (3, 2), (10, 11), (0, 1), (8, 9)],
    [(2, 3), (6, 7), (12, 13), (4, 5), (14, 15)],
    [(14, 15), (7, 6), (2, 3), (4, 5), (12, 13)],
    [(0, 1), (10, 11), (12, 13), (8, 9), (2, 3)],
    [(14, 15), (11, 10), (0, 1), (8, 9), (4, 5)],
    [(4, 5), (14, 15), (10, 11), (12, 13), (0, 1)],
    [(8, 9), (15, 14), (6, 7), (12, 13), (10, 11)],
]
# shared schedule: chain = (lhs_slot, m, rhs_slot)
CHAINS = ([(0, m, 2) for m in range(4)] + [(0, m, 3) for m in range(4)] +
          [(1, m, 2) for m in range(4)] + [(1, 0, 3), (1, 1, 3)] +
          [(4, m, 4) for m in range(4)])
CHAIN_GROUPS = [[0, 1, 2, 3], [4, 5, 6, 7], [8, 9, 10, 11], [12, 13],
                [14, 15, 16, 17]]
NCHAIN = len(CHAINS)


def _dma_order():
    """Input chunk order: first two chain-groups' slots chunk-interleaved,
    then remaining slots in first-use order."""
    first = []
    for ci in CHAIN_GROUPS[0] + CHAIN_GROUPS[1]:
        ls, _, rs = CHAINS[ci]
        for s in (ls, rs):
            if s not in first:
                first.append(s)
    rest = []
    for (ls, _, rs) in CHAINS:
        for s in (ls, rs):
            if s not in first and s not in rest:
                rest.append(s)
    order = []
    for c in range(NCHUNK):
        for s in first:
            order.append((s, c))
    for s in rest:
        for c in range(NCHUNK):
            order.append((s, c))
    return order


def _build_gram_kernel():
    import concourse.bacc as bacc
    import concourse.tile as tile
    from concourse import mybir

    nc = bacc.Bacc(None, target_bir_lowering=False)

    f32 = mybir.dt.float32
    bf16 = mybir.dt.bfloat16
    fp8 = mybir.dt.float8e4

    grps = nc.declare_dram_parameter("grps", [NSLOT, 128, KT, B], fp8,
                                     isOutput=False)
    out = nc.declare_dram_parameter("out", [NCHAIN, 128, B], bf16,
                                    isOutput=True)

    with tile.TileContext(nc) as tc:
        with (
            tc.tile_pool(name="grp_p", bufs=1) as grp_pool,
            tc.tile_pool(name="psum", bufs=8, space="PSUM") as psum_pool,
            tc.tile_pool(name="outp", bufs=6) as out_pool,
        ):
            gch = [[grp_pool.tile([128, NCHUNK, B], fp8, name=f"g{s}_{c}")
                    for c in range(NCHUNK)] for s in range(NSLOT)]

            for s, c in _dma_order():
                k0 = c * NCHUNK
                nc.sync.dma_start(gch[s][c][:], grps[s, :, k0:k0 + NCHUNK, :])

            for grp in CHAIN_GROUPS:
                pss = [psum_pool.tile([128, B], f32, name="ps") for _ in grp]
                for t in range(TT):
                    ct = t // 2
                    o = 2 * (t % 2)
                    for j, ci in enumerate(grp):
                        ls, m, rs = CHAINS[ci]
                        nc.tensor.matmul(
                            pss[j][:],
                            gch[ls][ct][:, o:o + 2, m * 128:(m + 1) * 128],
                            gch[rs][ct][:, o:o + 2, :],
                            start=(t == 0),
                            stop=(t == TT - 1),
                            perf_mode=mybir.MatmulPerfMode.DoubleRow,
                        )
                for j, ci in enumerate(grp):
                    ot = out_pool.tile([128, B], bf16, name="ot")
                    nc.vector.tensor_copy(ot[:], pss[j][:])
                    nc.scalar.dma_start(out[ci], ot[:])

    nc.compile()
    return nc


_NC_CACHE = None


def _pack_slot(eT8: np.ndarray, pair) -> np.ndarray:
    """eT8 [D, N] fp8 -> [128, KT, B] packed slot of two half-groups."""
    h0, h1 = pair
    blk = np.concatenate(
        [eT8[:, h0 * HALF:(h0 + 1) * HALF], eT8[:, h1 * HALF:(h1 + 1) * HALF]],
        axis=1)                                      # [2048, 512]
    return np.ascontiguousarray(
        blk.reshape(KT, 128, B).transpose(1, 0, 2))  # [128, 16, 512]


def _run_gram(emb: np.ndarray) -> np.ndarray:
    """Run the 8-core symmetric Gram kernel; returns G = emb @ emb.T f32."""
    global _NC_CACHE, LAST_EXEC_NS
    from concourse.bass_utils import run_bass_kernel_spmd

    if _NC_CACHE is None:
        _NC_CACHE = _build_gram_kernel()
    nc = _NC_CACHE

    eT8 = np.ascontiguousarray(emb.T).astype(ml_dtypes.float8_e4m3)
    pack_cache = {}
    in_maps = []
    for core in range(N_CORES):
        slabs = []
        for pair in SLOTPACK[core]:
            if pair not in pack_cache:
                pack_cache[pair] = _pack_slot(eT8, pair)
            slabs.append(pack_cache[pair])
        in_maps.append({"grps": np.ascontiguousarray(np.stack(slabs, axis=0))})

    trace = bool(int(os.environ.get("KERNEL_TRACE", "0")))
    res = run_bass_kernel_spmd(
        nc, in_maps, core_ids=list(range(N_CORES)), trace=trace
    )
    if res.exec_time_ns is not None:
        LAST_EXEC_NS = res.exec_time_ns

    G = np.empty((N, N), dtype=np.float32)
    for core in range(N_CORES):
        o = np.asarray(res.results[core]["out"], dtype=np.float32)  # [NCHAIN,128,B]
        S = SLOTPACK[core]
        for ci, (ls, m, rs) in enumerate(CHAINS):
            r0 = S[ls][m // 2] * HALF + (m % 2) * 128
            strip = o[ci]                       # [128, 512]
            for half in range(2):
                c0 = S[rs][half] * HALF
                piece = strip[:, half * HALF:(half + 1) * HALF]  # [128, 256]
                G[r0:r0 + 128, c0:c0 + HALF] = piece
                G[c0:c0 + HALF, r0:r0 + 128] = piece.T
    return G


def _sample_js(counts: np.ndarray, us: list) -> np.ndarray:
    """Replicate the reference's f32 sampling math. counts [N] int, us 3x[N]
    f32 uniforms. Returns j ranks [N, 3] int64 (rank into the masked sort)."""
    out = []
    for t, r in enumerate(RATIOS):
        cnt = np.maximum(
            np.int32(1),
            np.floor(counts.astype(np.float32) * np.float32(r)).astype(np.int32),
        )
        j = np.minimum((us[t] * cnt.astype(np.float32)).astype(np.int32), cnt - 1)
        out.append(j.astype(np.int64))
    return np.stack(out, axis=1)


def kernel(embeddings: np.ndarray, labels: np.ndarray) -> np.ndarray:
    emb = np.ascontiguousarray(np.asarray(embeddings, dtype=np.float32))
    lab = np.asarray(labels).astype(np.int64)

    G = _run_gram(emb)

    # Selection keys: within row i, ordering by (sq_j - 2 G[i,j]) equals
    # ordering by distance.
    sq = np.einsum("ij,ij->i", emb, emb).astype(np.float32)

    # Uniforms must match jax.random with key 42 bit-exactly.
    import jax

    with jax.default_device(jax.devices("cpu")[0]):
        skey = jax.random.key(42)
        keys = jax.random.split(skey, 6)
        us = [np.asarray(jax.random.uniform(k, (N,))) for k in keys]

    class_size = np.bincount(lab, minlength=NUM_IDS)
    pos_count = class_size[lab] - 1
    neg_count = N - class_size[lab]
    valid = (pos_count > 0) & (neg_count > 0)

    pos_js = _sample_js(pos_count, us[0:3])  # [N, 3]
    neg_js = _sample_js(neg_count, us[3:6])  # [N, 3]

    # Per-class member lists
    order = np.argsort(lab, kind="stable")
    sorted_lab = lab[order]
    starts = np.searchsorted(sorted_lab, np.arange(NUM_IDS), side="left")
    ends = np.searchsorted(sorted_lab, np.arange(NUM_IDS), side="right")

    pos_idx = np.zeros((N, 3), dtype=np.int64)
    neg_idx = np.zeros((N, 3), dtype=np.int64)
    INF = np.float32(np.inf)

    for i in range(N):
        li = lab[i]
        members = order[starts[li]:ends[li]]
        key_row = sq - 2.0 * G[i]  # f32 [N]
        if valid[i]:
            pos_members = members[members != i]
            pk = key_row[pos_members]
            po = np.argsort(pk, kind="stable")
            pos_idx[i] = pos_members[po[pos_js[i]]]
        # negatives: mask out own class and self
        nk = key_row.copy()
        nk[members] = INF
        nk[i] = INF
        kth = np.unique(neg_js[i])
        part = np.argpartition(nk, kth)
        neg_idx[i] = part[neg_js[i]]

    a = emb[:, None, :]
    p = emb[pos_idx]
    ng = emb[neg_idx]
    d_ap = np.sqrt(np.sum((a - p + np.float32(EPS)) ** 2, axis=-1))
    d_an = np.sqrt(np.sum((a - ng + np.float32(EPS)) ** 2, axis=-1))
    tri = np.maximum(d_ap - d_an + np.float32(MARGIN), np.float32(0.0))
    w = valid[:, None].astype(np.float32)
    denom = max(3.0 * float(valid.sum()), 1.0)
    loss = np.float32(np.sum(tri * w) / denom)
    return np.array(loss, dtype=np.float32)


# revision 12
# speedup vs baseline: 1.0854x; 1.0144x over previous
"""AdaptiveTripletLoss on 8 TRN2 NeuronCores.

Device: the compute-dominant Gram matrix G = E @ E^T in fp8 DoubleRow on
the PE, symmetry-aware (upper-triangular blocks only). Generic chain
machinery: each core loads NSLOT packed 512-column groups (each a pair
of 256-row half-groups, 4 chunks of 4 k-tiles per slot for fine-grained
DMA/compute overlap) and runs a fixed shared CHAINS schedule; the host
picks per-core slot contents so the union covers all of upper(G).
Dummy warm-up matmuls un-throttle the PE clock (HAM) while input DMAs
stream. Host mirrors blocks, then does masks/counts, order-statistic
selection, exact d_ap/d_an norms and the masked mean.
"""

import os

import numpy as np
import ml_dtypes

N, D = 4096, 2048
NUM_IDS = 512
N_CORES = 8
MARGIN = 0.3
RATIOS = (0.3, 0.4, 0.3)
EPS = 1e-6

B = 512           # block edge / slot width
HALF = 256        # half-group rows
KT = D // 128     # 16 k-tiles per slot
NCHUNK = 4        # 4 k-tiles per chunk
TT = KT // 2      # 8 DoubleRow steps per chain

LAST_EXEC_NS = None

# ---- cover definition (T6: 18 chains/core, 144 MMs) ----
# Half-group supers: super s = half-groups (2s, 2s+1). K16 minus the
# matching M equals K8 over supers with every edge blown up to K2,2;
# each core's cross coverage is the path P3-P0-P2-P1 (l-i-k-j) of a
# P4-decomposition of K8-F, the shared F-edge (j,l) is covered half by
# each core of a pair via the ordered P1 slot, and slot 4 is the core's
# own loop super (diag half-blocks).
NSLOT = 5
# SLOTPACK[core][slot] = (half-group, half-group): rows h*256..h*256+255
SLOTPACK = [
    [(4, 5), (2, 3), (8, 9), (0, 1), (6, 7)],
    [(6, 7), (3, 2), (10, 11), (0, 1), (8, 9)],
    [(2, 3), (6, 7), (12, 13), (4, 5), (14, 15)],
    [(14, 15), (7, 6), (2, 3), (4, 5), (12, 13)],
    [(0, 1), (10, 11), (12, 13), (8, 9), (2, 3)],
    [(14, 15), (11, 10), (0, 1), (8, 9), (4, 5)],
    [(4, 5), (14, 15), (10, 11), (12, 13), (0, 1)],
    [(8, 9), (15, 14), (6, 7), (12, 13), (10, 11)],
]
# shared schedule: chain = (lhs_slot, m, rhs_slot)
CHAINS = ([(0, m, 2) for m in range(4)] + [(0, m, 3) for m in range(4)] +
          [(1, m, 2) for m in range(4)] + [(1, 0, 3), (1, 1, 3)] +
          [(4, m, 4) for m in range(4)])
CHAIN_GROUPS = [[0, 1, 2, 3], [4, 5, 6, 7], [8, 9, 10, 11], [12, 13],
                [14, 15, 16, 17]]
NCHAIN = len(CHAINS)


def _dma_order():
    """Input chunk order: first two chain-groups' slots chunk-interleaved,
    then remaining slots in first-use order."""
    first = []
    for ci in CHAIN_GROUPS[0] + CHAIN_GROUPS[1]:
        ls, _, rs = CHAINS[ci]
        for s in (ls, rs):
            if s not in first:
                first.append(s)
    rest = []
    for (ls, _, rs) in CHAINS:
        for s in (ls, rs):
            if s not in first and s not in rest:
                rest.append(s)
    order = []
    for c in range(NCHUNK):
        for s in first:
            order.append((s, c))
    for s in rest:
        for c in range(NCHUNK):
            order.append((s, c))
    return order


def _build_gram_kernel():
    import concourse.bacc as bacc
    import concourse.tile as tile
    from concourse import mybir

    nc = bacc.Bacc(None, target_bir_lowering=False)

    f32 = mybir.dt.float32
    bf16 = mybir.dt.bfloat16
    fp8 = mybir.dt.float8e4

    grps = nc.declare_dram_parameter("grps", [NSLOT, 128, KT, B], fp8,
                                     isOutput=False)
    out = nc.declare_dram_parameter("out", [NCHAIN, 128, B], bf16,
                                    isOutput=True)

    with tile.TileContext(nc) as tc:
        with (
            tc.tile_pool(name="grp_p", bufs=1) as grp_pool,
            tc.tile_pool(name="psum", bufs=8, space="PSUM") as psum_pool,
            tc.tile_pool(name="outp", bufs=6) as out_pool,
        ):
            gch = [[grp_pool.tile([128, NCHUNK, B], fp8, name=f"g{s}_{c}")
                    for c in range(NCHUNK)] for s in range(NSLOT)]

            # First two critical chunks go out on the two parallel HWDGE
            # rings (SP + ACT) so their ~2us completion receipts overlap;
            # the rest stream on sync.
            order = _dma_order()
            (s0, c0), (s1, c1) = order[0], order[1]
            nc.sync.dma_start(gch[s0][c0][:],
                              grps[s0, :, c0 * NCHUNK:(c0 + 1) * NCHUNK, :])
            nc.scalar.dma_start(gch[s1][c1][:],
                                grps[s1, :, c1 * NCHUNK:(c1 + 1) * NCHUNK, :])
            for s, c in order[2:]:
                k0 = c * NCHUNK
                nc.sync.dma_start(gch[s][c][:], grps[s, :, k0:k0 + NCHUNK, :])

            for grp in CHAIN_GROUPS:
                pss = [psum_pool.tile([128, B], f32, name="ps") for _ in grp]
                for t in range(TT):
                    ct = t // 2
                    o = 2 * (t % 2)
                    for j, ci in enumerate(grp):
                        ls, m, rs = CHAINS[ci]
                        nc.tensor.matmul(
                            pss[j][:],
                            gch[ls][ct][:, o:o + 2, m * 128:(m + 1) * 128],
                            gch[rs][ct][:, o:o + 2, :],
                            start=(t == 0),
                            stop=(t == TT - 1),
                            perf_mode=mybir.MatmulPerfMode.DoubleRow,
                        )
                for j, ci in enumerate(grp):
                    ot = out_pool.tile([128, B], bf16, name="ot")
                    # PSUM->SBUF casts alternate DVE/ACT (parallel PSUM
                    # ports); output DMAs ride the sync ring, which is
                    # done issuing inputs by the time casts land.
                    if ci % 2 == 0:
                        nc.vector.tensor_copy(ot[:], pss[j][:])
                    else:
                        nc.scalar.copy(ot[:], pss[j][:])
                    nc.sync.dma_start(out[ci], ot[:])

    nc.compile()
    return nc


_NC_CACHE = None


def _pack_slot(eT8: np.ndarray, pair) -> np.ndarray:
    """eT8 [D, N] fp8 -> [128, KT, B] packed slot of two half-groups."""
    h0, h1 = pair
    blk = np.concatenate(
        [eT8[:, h0 * HALF:(h0 + 1) * HALF], eT8[:, h1 * HALF:(h1 + 1) * HALF]],
        axis=1)                                      # [2048, 512]
    return np.ascontiguousarray(
        blk.reshape(KT, 128, B).transpose(1, 0, 2))  # [128, 16, 512]


def _run_gram(emb: np.ndarray) -> np.ndarray:
    """Run the 8-core symmetric Gram kernel; returns G = emb @ emb.T f32."""
    global _NC_CACHE, LAST_EXEC_NS
    from concourse.bass_utils import run_bass_kernel_spmd

    if _NC_CACHE is None:
        _NC_CACHE = _build_gram_kernel()
    nc = _NC_CACHE

    eT8 = np.ascontiguousarray(emb.T).astype(ml_dtypes.float8_e4m3)
    pack_cache = {}
    in_maps = []
    for core in range(N_CORES):
        slabs = []
        for pair in SLOTPACK[core]:
            if pair not in pack_cache:
                pack_cache[pair] = _pack_slot(eT8, pair)
            slabs.append(pack_cache[pair])
        in_maps.append({"grps": np.ascontiguousarray(np.stack(slabs, axis=0))})

    trace = bool(int(os.environ.get("KERNEL_TRACE", "0")))
    res = run_bass_kernel_spmd(
        nc, in_maps, core_ids=list(range(N_CORES)), trace=trace
    )
    if res.exec_time_ns is not None:
        LAST_EXEC_NS = res.exec_time_ns

    G = np.empty((N, N), dtype=np.float32)
    for core in range(N_CORES):
        o = np.asarray(res.results[core]["out"], dtype=np.float32)  # [NCHAIN,128,B]
        S = SLOTPACK[core]
        for ci, (ls, m, rs) in enumerate(CHAINS):
            r0 = S[ls][m // 2] * HALF + (m % 2) * 128
            strip = o[ci]                       # [128, 512]
            for half in range(2):
                c0 = S[rs][half] * HALF
                piece = strip[:, half * HALF:(half + 1) * HALF]  # [128, 256]
                G[r0:r0 + 128, c0:c0 + HALF] = piece
                G[c0:c0 + HALF, r0:r0 + 128] = piece.T
    return G


def _sample_js(counts: np.ndarray, us: list) -> np.ndarray:
    """Replicate the reference's f32 sampling math. counts [N] int, us 3x[N]
    f32 uniforms. Returns j ranks [N, 3] int64 (rank into the masked sort)."""
    out = []
    for t, r in enumerate(RATIOS):
        cnt = np.maximum(
            np.int32(1),
            np.floor(counts.astype(np.float32) * np.float32(r)).astype(np.int32),
        )
        j = np.minimum((us[t] * cnt.astype(np.float32)).astype(np.int32), cnt - 1)
        out.append(j.astype(np.int64))
    return np.stack(out, axis=1)


def kernel(embeddings: np.ndarray, labels: np.ndarray) -> np.ndarray:
    emb = np.ascontiguousarray(np.asarray(embeddings, dtype=np.float32))
    lab = np.asarray(labels).astype(np.int64)

    G = _run_gram(emb)

    # Selection keys: within row i, ordering by (sq_j - 2 G[i,j]) equals
    # ordering by distance.
    sq = np.einsum("ij,ij->i", emb, emb).astype(np.float32)

    # Uniforms must match jax.random with key 42 bit-exactly.
    import jax

    with jax.default_device(jax.devices("cpu")[0]):
        skey = jax.random.key(42)
        keys = jax.random.split(skey, 6)
        us = [np.asarray(jax.random.uniform(k, (N,))) for k in keys]

    class_size = np.bincount(lab, minlength=NUM_IDS)
    pos_count = class_size[lab] - 1
    neg_count = N - class_size[lab]
    valid = (pos_count > 0) & (neg_count > 0)

    pos_js = _sample_js(pos_count, us[0:3])  # [N, 3]
    neg_js = _sample_js(neg_count, us[3:6])  # [N, 3]

    # Per-class member lists
    order = np.argsort(lab, kind="stable")
    sorted_lab = lab[order]
    starts = np.searchsorted(sorted_lab, np.arange(NUM_IDS), side="left")
    ends = np.searchsorted(sorted_lab, np.arange(NUM_IDS), side="right")

    pos_idx = np.zeros((N, 3), dtype=np.int64)
    neg_idx = np.zeros((N, 3), dtype=np.int64)
    INF = np.float32(np.inf)

    for i in range(N):
        li = lab[i]
        members = order[starts[li]:ends[li]]
        key_row = sq - 2.0 * G[i]  # f32 [N]
        if valid[i]:
            pos_members = members[members != i]
            pk = key_row[pos_members]
            po = np.argsort(pk, kind="stable")
            pos_idx[i] = pos_members[po[pos_js[i]]]
        # negatives: mask out own class and self
        nk = key_row.copy()
        nk[members] = INF
        nk[i] = INF
        kth = np.unique(neg_js[i])
        part = np.argpartition(nk, kth)
        neg_idx[i] = part[neg_js[i]]

    a = emb[:, None, :]
    p = emb[pos_idx]
    ng = emb[neg_idx]
    d_ap = np.sqrt(np.sum((a - p + np.float32(EPS)) ** 2, axis=-1))
    d_an = np.sqrt(np.sum((a - ng + np.float32(EPS)) ** 2, axis=-1))
    tri = np.maximum(d_ap - d_an + np.float32(MARGIN), np.float32(0.0))
    w = valid[:, None].astype(np.float32)
    denom = max(3.0 * float(valid.sum()), 1.0)
    loss = np.float32(np.sum(tri * w) / denom)
    return np.array(loss, dtype=np.float32)


# revision 15
# speedup vs baseline: 1.1200x; 1.0319x over previous
"""AdaptiveTripletLoss on 8 TRN2 NeuronCores.

Device: the compute-dominant Gram matrix G = E @ E^T in fp8 DoubleRow on
the PE, symmetry-aware (upper-triangular blocks only). Generic chain
machinery: each core loads NSLOT packed 512-column groups (each a pair
of 256-row half-groups, 4 chunks of 4 k-tiles per slot for fine-grained
DMA/compute overlap) and runs a fixed shared CHAINS schedule; the host
picks per-core slot contents so the union covers all of upper(G).
Dummy warm-up matmuls un-throttle the PE clock (HAM) while input DMAs
stream. Host mirrors blocks, then does masks/counts, order-statistic
selection, exact d_ap/d_an norms and the masked mean.
"""

import os

import numpy as np
import ml_dtypes

N, D = 4096, 2048
NUM_IDS = 512
N_CORES = 8
MARGIN = 0.3
RATIOS = (0.3, 0.4, 0.3)
EPS = 1e-6

B = 512           # block edge / slot width
HALF = 256        # half-group rows
KT = D // 128     # 16 k-tiles per slot
NCHUNK = 4        # 4 k-tiles per chunk
TT = KT // 2      # 8 DoubleRow steps per chain

LAST_EXEC_NS = None

# ---- cover definition (T6: 18 chains/core, 144 MMs) ----
# Half-group supers: super s = half-groups (2s, 2s+1). K16 minus the
# matching M equals K8 over supers with every edge blown up to K2,2;
# each core's cross coverage is the path P3-P0-P2-P1 (l-i-k-j) of a
# P4-decomposition of K8-F, the shared F-edge (j,l) is covered half by
# each core of a pair via the ordered P1 slot, and slot 4 is the core's
# own loop super (diag half-blocks).
NSLOT = 5
# SLOTPACK[core][slot] = (half-group, half-group): rows h*256..h*256+255
SLOTPACK = [
    [(4, 5), (2, 3), (8, 9), (0, 1), (6, 7)],
    [(6, 7), (3, 2), (10, 11), (0, 1), (8, 9)],
    [(2, 3), (6, 7), (12, 13), (4, 5), (14, 15)],
    [(14, 15), (7, 6), (2, 3), (4, 5), (12, 13)],
    [(0, 1), (10, 11), (12, 13), (8, 9), (2, 3)],
    [(14, 15), (11, 10), (0, 1), (8, 9), (4, 5)],
    [(4, 5), (14, 15), (10, 11), (12, 13), (0, 1)],
    [(8, 9), (15, 14), (6, 7), (12, 13), (10, 11)],
]
# shared schedule: chain = (lhs_slot, m, rhs_slot)
CHAINS = ([(0, m, 2) for m in range(4)] + [(0, m, 3) for m in range(4)] +
          [(1, m, 2) for m in range(4)] + [(1, 0, 3), (1, 1, 3)] +
          [(4, m, 4) for m in range(4)])
CHAIN_GROUPS = [[0, 1, 2, 3], [4, 5, 6, 7], [8, 9, 10, 11], [12, 13],
                [14, 15, 16, 17]]
NCHAIN = len(CHAINS)


def _dma_order():
    """Input chunk order: first two chain-groups' slots chunk-interleaved,
    then remaining slots in first-use order."""
    first = []
    for ci in CHAIN_GROUPS[0] + CHAIN_GROUPS[1]:
        ls, _, rs = CHAINS[ci]
        for s in (ls, rs):
            if s not in first:
                first.append(s)
    rest = []
    for (ls, _, rs) in CHAINS:
        for s in (ls, rs):
            if s not in first and s not in rest:
                rest.append(s)
    order = []
    for c in range(NCHUNK):
        for s in first:
            order.append((s, c))
    for s in rest:
        for c in range(NCHUNK):
            order.append((s, c))
    return order


def _build_gram_kernel():
    import concourse.bacc as bacc
    import concourse.tile as tile
    from concourse import mybir

    nc = bacc.Bacc(None, target_bir_lowering=False)

    f32 = mybir.dt.float32
    bf16 = mybir.dt.bfloat16
    fp8 = mybir.dt.float8e4

    grps = nc.declare_dram_parameter("grps", [NSLOT, 128, KT, B], fp8,
                                     isOutput=False)
    out = nc.declare_dram_parameter("out", [NCHAIN, 128, B], bf16,
                                    isOutput=True)

    with tile.TileContext(nc) as tc:
        with (
            tc.tile_pool(name="grp_p", bufs=1) as grp_pool,
            tc.tile_pool(name="psum", bufs=8, space="PSUM") as psum_pool,
            tc.tile_pool(name="outp", bufs=6) as out_pool,
        ):
            gch = [[grp_pool.tile([128, NCHUNK, B], fp8, name=f"g{s}_{c}")
                    for c in range(NCHUNK)] for s in range(NSLOT)]
            dmy = grp_pool.tile([128, 2, B], fp8, name="dmy")

            # First two critical chunks go out on the two parallel HWDGE
            # rings (SP + ACT) so their ~2us completion receipts overlap;
            # the rest stream on sync.
            order = _dma_order()
            (s0, c0), (s1, c1) = order[0], order[1]
            nc.sync.dma_start(gch[s0][c0][:],
                              grps[s0, :, c0 * NCHUNK:(c0 + 1) * NCHUNK, :])
            nc.scalar.dma_start(gch[s1][c1][:],
                                grps[s1, :, c1 * NCHUNK:(c1 + 1) * NCHUNK, :])
            for s, c in order[2:]:
                k0 = c * NCHUNK
                nc.sync.dma_start(gch[s][c][:], grps[s, :, k0:k0 + NCHUNK, :])

            # PE warm-up while the first chunks' ~2.5us HBM receipt is in
            # flight: 7 cold dummy matmuls (~8.2-11.2us) hold the HAM
            # activity window so the real chains start at full clock.
            nc.gpsimd.memset(dmy[:], 0.0)
            for i in range(7):
                wp = psum_pool.tile([128, B], f32, name="ps")
                nc.tensor.matmul(
                    wp[:], dmy[:, :, 0:128], dmy[:],
                    start=True, stop=True,
                    perf_mode=mybir.MatmulPerfMode.DoubleRow,
                )

            for grp in CHAIN_GROUPS:
                pss = [psum_pool.tile([128, B], f32, name="ps") for _ in grp]
                for t in range(TT):
                    ct = t // 2
                    o = 2 * (t % 2)
                    for j, ci in enumerate(grp):
                        ls, m, rs = CHAINS[ci]
                        nc.tensor.matmul(
                            pss[j][:],
                            gch[ls][ct][:, o:o + 2, m * 128:(m + 1) * 128],
                            gch[rs][ct][:, o:o + 2, :],
                            start=(t == 0),
                            stop=(t == TT - 1),
                            perf_mode=mybir.MatmulPerfMode.DoubleRow,
                        )
                for j, ci in enumerate(grp):
                    ot = out_pool.tile([128, B], bf16, name="ot")
                    # PSUM->SBUF casts alternate DVE/ACT (parallel PSUM
                    # ports); each chain's output DMA rides the other
                    # HWDGE ring than its cast engine so the tail
                    # parallelizes.
                    if ci % 2 == 0:
                        nc.vector.tensor_copy(ot[:], pss[j][:])
                        nc.scalar.dma_start(out[ci], ot[:])
                    else:
                        nc.scalar.copy(ot[:], pss[j][:])
                        nc.sync.dma_start(out[ci], ot[:])

    nc.compile()
    return nc


_NC_CACHE = None


def _pack_slot(eT8: np.ndarray, pair) -> np.ndarray:
    """eT8 [D, N] fp8 -> [128, KT, B] packed slot of two half-groups."""
    h0, h1 = pair
    blk = np.concatenate(
        [eT8[:, h0 * HALF:(h0 + 1) * HALF], eT8[:, h1 * HALF:(h1 + 1) * HALF]],
        axis=1)                                      # [2048, 512]
    return np.ascontiguousarray(
        blk.reshape(KT, 128, B).transpose(1, 0, 2))  # [128, 16, 512]


def _run_gram(emb: np.ndarray) -> np.ndarray:
    """Run the 8-core symmetric Gram kernel; returns G = emb @ emb.T f32."""
    global _NC_CACHE, LAST_EXEC_NS
    from concourse.bass_utils import run_bass_kernel_spmd

    if _NC_CACHE is None:
        _NC_CACHE = _build_gram_kernel()
    nc = _NC_CACHE

    eT8 = np.ascontiguousarray(emb.T).astype(ml_dtypes.float8_e4m3)
    pack_cache = {}
    in_maps = []
    for core in range(N_CORES):
        slabs = []
        for pair in SLOTPACK[core]:
            if pair not in pack_cache:
                pack_cache[pair] = _pack_slot(eT8, pair)
            slabs.append(pack_cache[pair])
        in_maps.append({"grps": np.ascontiguousarray(np.stack(slabs, axis=0))})

    trace = bool(int(os.environ.get("KERNEL_TRACE", "0")))
    res = run_bass_kernel_spmd(
        nc, in_maps, core_ids=list(range(N_CORES)), trace=trace
    )
    if res.exec_time_ns is not None:
        LAST_EXEC_NS = res.exec_time_ns

    G = np.empty((N, N), dtype=np.float32)
    for core in range(N_CORES):
        o = np.asarray(res.results[core]["out"], dtype=np.float32)  # [NCHAIN,128,B]
        S = SLOTPACK[core]
        for ci, (ls, m, rs) in enumerate(CHAINS):
            r0 = S[ls][m // 2] * HALF + (m % 2) * 128
            strip = o[ci]                       # [128, 512]
            for half in range(2):
                c0 = S[rs][half] * HALF
                piece = strip[:, half * HALF:(half + 1) * HALF]  # [128, 256]
                G[r0:r0 + 128, c0:c0 + HALF] = piece
                G[c0:c0 + HALF, r0:r0 + 128] = piece.T
    return G


def _sample_js(counts: np.ndarray, us: list) -> np.ndarray:
    """Replicate the reference's f32 sampling math. counts [N] int, us 3x[N]
    f32 uniforms. Returns j ranks [N, 3] int64 (rank into the masked sort)."""
    out = []
    for t, r in enumerate(RATIOS):
        cnt = np.maximum(
            np.int32(1),
            np.floor(counts.astype(np.float32) * np.float32(r)).astype(np.int32),
        )
        j = np.minimum((us[t] * cnt.astype(np.float32)).astype(np.int32), cnt - 1)
        out.append(j.astype(np.int64))
    return np.stack(out, axis=1)


def kernel(embeddings: np.ndarray, labels: np.ndarray) -> np.ndarray:
    emb = np.ascontiguousarray(np.asarray(embeddings, dtype=np.float32))
    lab = np.asarray(labels).astype(np.int64)

    G = _run_gram(emb)

    # Selection keys: within row i, ordering by (sq_j - 2 G[i,j]) equals
    # ordering by distance.
    sq = np.einsum("ij,ij->i", emb, emb).astype(np.float32)

    # Uniforms must match jax.random with key 42 bit-exactly.
    import jax

    with jax.default_device(jax.devices("cpu")[0]):
        skey = jax.random.key(42)
        keys = jax.random.split(skey, 6)
        us = [np.asarray(jax.random.uniform(k, (N,))) for k in keys]

    class_size = np.bincount(lab, minlength=NUM_IDS)
    pos_count = class_size[lab] - 1
    neg_count = N - class_size[lab]
    valid = (pos_count > 0) & (neg_count > 0)

    pos_js = _sample_js(pos_count, us[0:3])  # [N, 3]
    neg_js = _sample_js(neg_count, us[3:6])  # [N, 3]

    # Per-class member lists
    order = np.argsort(lab, kind="stable")
    sorted_lab = lab[order]
    starts = np.searchsorted(sorted_lab, np.arange(NUM_IDS), side="left")
    ends = np.searchsorted(sorted_lab, np.arange(NUM_IDS), side="right")

    pos_idx = np.zeros((N, 3), dtype=np.int64)
    neg_idx = np.zeros((N, 3), dtype=np.int64)
    INF = np.float32(np.inf)

    for i in range(N):
        li = lab[i]
        members = order[starts[li]:ends[li]]
        key_row = sq - 2.0 * G[i]  # f32 [N]
        if valid[i]:
            pos_members = members[members != i]
            pk = key_row[pos_members]
            po = np.argsort(pk, kind="stable")
            pos_idx[i] = pos_members[po[pos_js[i]]]
        # negatives: mask out own class and self
        nk = key_row.copy()
        nk[members] = INF
        nk[i] = INF
        kth = np.unique(neg_js[i])
        part = np.argpartition(nk, kth)
        neg_idx[i] = part[neg_js[i]]

    a = emb[:, None, :]
    p = emb[pos_idx]
    ng = emb[neg_idx]
    d_ap = np.sqrt(np.sum((a - p + np.float32(EPS)) ** 2, axis=-1))
    d_an = np.sqrt(np.sum((a - ng + np.float32(EPS)) ** 2, axis=-1))
    tri = np.maximum(d_ap - d_an + np.float32(MARGIN), np.float32(0.0))
    w = valid[:, None].astype(np.float32)
    denom = max(3.0 * float(valid.sum()), 1.0)
    loss = np.float32(np.sum(tri * w) / denom)
    return np.array(loss, dtype=np.float32)


# revision 17
# speedup vs baseline: 1.1526x; 1.0291x over previous
"""AdaptiveTripletLoss on 8 TRN2 NeuronCores.

Device: the compute-dominant Gram matrix G = E @ E^T in fp8 DoubleRow on
the PE, symmetry-aware (upper-triangular blocks only). Generic chain
machinery: each core loads NSLOT packed 512-column groups (each a pair
of 256-row half-groups, 4 chunks of 4 k-tiles per slot for fine-grained
DMA/compute overlap) and runs a fixed shared CHAINS schedule; the host
picks per-core slot contents so the union covers all of upper(G).
Dummy warm-up matmuls un-throttle the PE clock (HAM) while input DMAs
stream. Host mirrors blocks, then does masks/counts, order-statistic
selection, exact d_ap/d_an norms and the masked mean.
"""

import os

import numpy as np
import ml_dtypes

N, D = 4096, 2048
NUM_IDS = 512
N_CORES = 8
MARGIN = 0.3
RATIOS = (0.3, 0.4, 0.3)
EPS = 1e-6

B = 512           # block edge / slot width
HALF = 256        # half-group rows
KT = D // 128     # 16 k-tiles per slot
NCHUNK = 4        # 4 k-tiles per chunk
TT = KT // 2      # 8 DoubleRow steps per chain

LAST_EXEC_NS = None

# ---- cover definition (T6: 18 chains/core, 144 MMs) ----
# Half-group supers: super s = half-groups (2s, 2s+1). K16 minus the
# matching M equals K8 over supers with every edge blown up to K2,2;
# each core's cross coverage is the path P3-P0-P2-P1 (l-i-k-j) of a
# P4-decomposition of K8-F, the shared F-edge (j,l) is covered half by
# each core of a pair via the ordered P1 slot, and slot 4 is the core's
# own loop super (diag half-blocks).
NSLOT = 5
# SLOTPACK[core][slot] = (half-group, half-group): rows h*256..h*256+255
SLOTPACK = [
    [(4, 5), (2, 3), (8, 9), (0, 1), (6, 7)],
    [(6, 7), (3, 2), (10, 11), (0, 1), (8, 9)],
    [(2, 3), (6, 7), (12, 13), (4, 5), (14, 15)],
    [(14, 15), (7, 6), (2, 3), (4, 5), (12, 13)],
    [(0, 1), (10, 11), (12, 13), (8, 9), (2, 3)],
    [(14, 15), (11, 10), (0, 1), (8, 9), (4, 5)],
    [(4, 5), (14, 15), (10, 11), (12, 13), (0, 1)],
    [(8, 9), (15, 14), (6, 7), (12, 13), (10, 11)],
]
# shared schedule: chain = (lhs_slot, m, rhs_slot)
CHAINS = ([(0, m, 2) for m in range(4)] + [(0, m, 3) for m in range(4)] +
          [(1, m, 2) for m in range(4)] + [(1, 0, 3), (1, 1, 3)] +
          [(4, m, 4) for m in range(4)])
CHAIN_GROUPS = [[0, 1, 2, 3], [4, 5, 6, 7], [8, 9, 10, 11],
                [14, 15, 16, 17], [12, 13]]
NCHAIN = len(CHAINS)


def _dma_order():
    """Input chunk order = strict consumption order: the first group's
    slots chunk-interleaved, then each later group's new slots in group
    order (warm MMs consume exactly at delivery rate, so any chunk
    loaded ahead of its use steals bandwidth from the active group)."""
    first = []
    for ci in CHAIN_GROUPS[0]:
        ls, _, rs = CHAINS[ci]
        for s in (ls, rs):
            if s not in first:
                first.append(s)
    rest = []
    for grp in CHAIN_GROUPS[1:]:
        for ci in grp:
            ls, _, rs = CHAINS[ci]
            for s in (ls, rs):
                if s not in first and s not in rest:
                    rest.append(s)
    order = []
    for c in range(NCHUNK):
        for s in first:
            order.append((s, c))
    for s in rest:
        for c in range(NCHUNK):
            order.append((s, c))
    return order


def _build_gram_kernel():
    import concourse.bacc as bacc
    import concourse.tile as tile
    from concourse import mybir

    nc = bacc.Bacc(None, target_bir_lowering=False)

    f32 = mybir.dt.float32
    bf16 = mybir.dt.bfloat16
    fp8 = mybir.dt.float8e4

    grps = nc.declare_dram_parameter("grps", [NSLOT, 128, KT, B], fp8,
                                     isOutput=False)
    out = nc.declare_dram_parameter("out", [NCHAIN, 128, B], bf16,
                                    isOutput=True)

    with tile.TileContext(nc) as tc:
        with (
            tc.tile_pool(name="grp_p", bufs=1) as grp_pool,
            tc.tile_pool(name="psum", bufs=8, space="PSUM") as psum_pool,
            tc.tile_pool(name="outp", bufs=6) as out_pool,
        ):
            gch = [[grp_pool.tile([128, NCHUNK, B], fp8, name=f"g{s}_{c}")
                    for c in range(NCHUNK)] for s in range(NSLOT)]
            dmy = grp_pool.tile([128, 2, B], fp8, name="dmy")

            # First two critical chunks go out on the two parallel HWDGE
            # rings (SP + ACT) so their ~2us completion receipts overlap;
            # the rest stream on sync.
            order = _dma_order()
            (s0, c0), (s1, c1) = order[0], order[1]
            nc.sync.dma_start(gch[s0][c0][:],
                              grps[s0, :, c0 * NCHUNK:(c0 + 1) * NCHUNK, :])
            nc.scalar.dma_start(gch[s1][c1][:],
                                grps[s1, :, c1 * NCHUNK:(c1 + 1) * NCHUNK, :])
            for s, c in order[2:]:
                k0 = c * NCHUNK
                nc.sync.dma_start(gch[s][c][:], grps[s, :, k0:k0 + NCHUNK, :])

            # PE warm-up while the first chunks' ~2.5us HBM receipt is in
            # flight: 7 cold dummy matmuls (~8.2-11.2us) hold the HAM
            # activity window so the real chains start at full clock.
            nc.gpsimd.memset(dmy[:], 0.0)
            for i in range(7):
                wp = psum_pool.tile([128, B], f32, name="ps")
                nc.tensor.matmul(
                    wp[:], dmy[:, :, 0:128], dmy[:],
                    start=True, stop=True,
                    perf_mode=mybir.MatmulPerfMode.DoubleRow,
                )

            for grp in CHAIN_GROUPS:
                pss = [psum_pool.tile([128, B], f32, name="ps") for _ in grp]
                for t in range(TT):
                    ct = t // 2
                    o = 2 * (t % 2)
                    for j, ci in enumerate(grp):
                        ls, m, rs = CHAINS[ci]
                        nc.tensor.matmul(
                            pss[j][:],
                            gch[ls][ct][:, o:o + 2, m * 128:(m + 1) * 128],
                            gch[rs][ct][:, o:o + 2, :],
                            start=(t == 0),
                            stop=(t == TT - 1),
                            perf_mode=mybir.MatmulPerfMode.DoubleRow,
                        )
                for j, ci in enumerate(grp):
                    ot = out_pool.tile([128, B], bf16, name="ot")
                    # PSUM->SBUF casts alternate DVE/ACT (parallel PSUM
                    # ports); each chain's output DMA rides the other
                    # HWDGE ring than its cast engine so the tail
                    # parallelizes.
                    if ci % 2 == 0:
                        nc.vector.tensor_copy(ot[:], pss[j][:])
                        nc.scalar.dma_start(out[ci], ot[:])
                    else:
                        nc.scalar.copy(ot[:], pss[j][:])
                        nc.sync.dma_start(out[ci], ot[:])

    nc.compile()
    return nc


_NC_CACHE = None


def _pack_slot(eT8: np.ndarray, pair) -> np.ndarray:
    """eT8 [D, N] fp8 -> [128, KT, B] packed slot of two half-groups."""
    h0, h1 = pair
    blk = np.concatenate(
        [eT8[:, h0 * HALF:(h0 + 1) * HALF], eT8[:, h1 * HALF:(h1 + 1) * HALF]],
        axis=1)                                      # [2048, 512]
    return np.ascontiguousarray(
        blk.reshape(KT, 128, B).transpose(1, 0, 2))  # [128, 16, 512]


def _run_gram(emb: np.ndarray) -> np.ndarray:
    """Run the 8-core symmetric Gram kernel; returns G = emb @ emb.T f32."""
    global _NC_CACHE, LAST_EXEC_NS
    from concourse.bass_utils import run_bass_kernel_spmd

    if _NC_CACHE is None:
        _NC_CACHE = _build_gram_kernel()
    nc = _NC_CACHE

    eT8 = np.ascontiguousarray(emb.T).astype(ml_dtypes.float8_e4m3)
    pack_cache = {}
    in_maps = []
    for core in range(N_CORES):
        slabs = []
        for pair in SLOTPACK[core]:
            if pair not in pack_cache:
                pack_cache[pair] = _pack_slot(eT8, pair)
            slabs.append(pack_cache[pair])
        in_maps.append({"grps": np.ascontiguousarray(np.stack(slabs, axis=0))})

    trace = bool(int(os.environ.get("KERNEL_TRACE", "0")))
    res = run_bass_kernel_spmd(
        nc, in_maps, core_ids=list(range(N_CORES)), trace=trace
    )
    if res.exec_time_ns is not None:
        LAST_EXEC_NS = res.exec_time_ns

    G = np.empty((N, N), dtype=np.float32)
    for core in range(N_CORES):
        o = np.asarray(res.results[core]["out"], dtype=np.float32)  # [NCHAIN,128,B]
        S = SLOTPACK[core]
        for ci, (ls, m, rs) in enumerate(CHAINS):
            r0 = S[ls][m // 2] * HALF + (m % 2) * 128
            strip = o[ci]                       # [128, 512]
            for half in range(2):
                c0 = S[rs][half] * HALF
                piece = strip[:, half * HALF:(half + 1) * HALF]  # [128, 256]
                G[r0:r0 + 128, c0:c0 + HALF] = piece
                G[c0:c0 + HALF, r0:r0 + 128] = piece.T
    return G


def _sample_js(counts: np.ndarray, us: list) -> np.ndarray:
    """Replicate the reference's f32 sampling math. counts [N] int, us 3x[N]
    f32 uniforms. Returns j ranks [N, 3] int64 (rank into the masked sort)."""
    out = []
    for t, r in enumerate(RATIOS):
        cnt = np.maximum(
            np.int32(1),
            np.floor(counts.astype(np.float32) * np.float32(r)).astype(np.int32),
        )
        j = np.minimum((us[t] * cnt.astype(np.float32)).astype(np.int32), cnt - 1)
        out.append(j.astype(np.int64))
    return np.stack(out, axis=1)


def kernel(embeddings: np.ndarray, labels: np.ndarray) -> np.ndarray:
    emb = np.ascontiguousarray(np.asarray(embeddings, dtype=np.float32))
    lab = np.asarray(labels).astype(np.int64)

    G = _run_gram(emb)

    # Selection keys: within row i, ordering by (sq_j - 2 G[i,j]) equals
    # ordering by distance.
    sq = np.einsum("ij,ij->i", emb, emb).astype(np.float32)

    # Uniforms must match jax.random with key 42 bit-exactly.
    import jax

    with jax.default_device(jax.devices("cpu")[0]):
        skey = jax.random.key(42)
        keys = jax.random.split(skey, 6)
        us = [np.asarray(jax.random.uniform(k, (N,))) for k in keys]

    class_size = np.bincount(lab, minlength=NUM_IDS)
    pos_count = class_size[lab] - 1
    neg_count = N - class_size[lab]
    valid = (pos_count > 0) & (neg_count > 0)

    pos_js = _sample_js(pos_count, us[0:3])  # [N, 3]
    neg_js = _sample_js(neg_count, us[3:6])  # [N, 3]

    # Per-class member lists
    order = np.argsort(lab, kind="stable")
    sorted_lab = lab[order]
    starts = np.searchsorted(sorted_lab, np.arange(NUM_IDS), side="left")
    ends = np.searchsorted(sorted_lab, np.arange(NUM_IDS), side="right")

    pos_idx = np.zeros((N, 3), dtype=np.int64)
    neg_idx = np.zeros((N, 3), dtype=np.int64)
    INF = np.float32(np.inf)

    for i in range(N):
        li = lab[i]
        members = order[starts[li]:ends[li]]
        key_row = sq - 2.0 * G[i]  # f32 [N]
        if valid[i]:
            pos_members = members[members != i]
            pk = key_row[pos_members]
            po = np.argsort(pk, kind="stable")
            pos_idx[i] = pos_members[po[pos_js[i]]]
        # negatives: mask out own class and self
        nk = key_row.copy()
        nk[members] = INF
        nk[i] = INF
        kth = np.unique(neg_js[i])
        part = np.argpartition(nk, kth)
        neg_idx[i] = part[neg_js[i]]

    a = emb[:, None, :]
    p = emb[pos_idx]
    ng = emb[neg_idx]
    d_ap = np.sqrt(np.sum((a - p + np.float32(EPS)) ** 2, axis=-1))
    d_an = np.sqrt(np.sum((a - ng + np.float32(EPS)) ** 2, axis=-1))
    tri = np.maximum(d_ap - d_an + np.float32(MARGIN), np.float32(0.0))
    w = valid[:, None].astype(np.float32)
    denom = max(3.0 * float(valid.sum()), 1.0)
    loss = np.float32(np.sum(tri * w) / denom)
    return np.array(loss, dtype=np.float32)


# revision 18
# speedup vs baseline: 1.1531x; 1.0005x over previous
"""AdaptiveTripletLoss on 8 TRN2 NeuronCores.

Device: the compute-dominant Gram matrix G = E @ E^T in fp8 DoubleRow on
the PE, symmetry-aware (upper-triangular blocks only). Generic chain
machinery: each core loads NSLOT packed 512-column groups (each a pair
of 256-row half-groups, 4 chunks of 4 k-tiles per slot for fine-grained
DMA/compute overlap) and runs a fixed shared CHAINS schedule; the host
picks per-core slot contents so the union covers all of upper(G).
Dummy warm-up matmuls un-throttle the PE clock (HAM) while input DMAs
stream. Host mirrors blocks, then does masks/counts, order-statistic
selection, exact d_ap/d_an norms and the masked mean.
"""

import os

import numpy as np
import ml_dtypes

N, D = 4096, 2048
NUM_IDS = 512
N_CORES = 8
MARGIN = 0.3
RATIOS = (0.3, 0.4, 0.3)
EPS = 1e-6

B = 512           # block edge / slot width
HALF = 256        # half-group rows
KT = D // 128     # 16 k-tiles per slot
NCHUNK = 4        # 4 k-tiles per chunk
TT = KT // 2      # 8 DoubleRow steps per chain

LAST_EXEC_NS = None

# ---- cover definition (T6: 18 chains/core, 144 MMs) ----
# Half-group supers: super s = half-groups (2s, 2s+1). K16 minus the
# matching M equals K8 over supers with every edge blown up to K2,2;
# each core's cross coverage is the path P3-P0-P2-P1 (l-i-k-j) of a
# P4-decomposition of K8-F, the shared F-edge (j,l) is covered half by
# each core of a pair via the ordered P1 slot, and slot 4 is the core's
# own loop super (diag half-blocks).
NSLOT = 5
# SLOTPACK[core][slot] = (half-group, half-group): rows h*256..h*256+255
SLOTPACK = [
    [(4, 5), (2, 3), (8, 9), (0, 1), (6, 7)],
    [(6, 7), (3, 2), (10, 11), (0, 1), (8, 9)],
    [(2, 3), (6, 7), (12, 13), (4, 5), (14, 15)],
    [(14, 15), (7, 6), (2, 3), (4, 5), (12, 13)],
    [(0, 1), (10, 11), (12, 13), (8, 9), (2, 3)],
    [(14, 15), (11, 10), (0, 1), (8, 9), (4, 5)],
    [(4, 5), (14, 15), (10, 11), (12, 13), (0, 1)],
    [(8, 9), (15, 14), (6, 7), (12, 13), (10, 11)],
]
# shared schedule: chain = (lhs_slot, m, rhs_slot)
CHAINS = ([(0, m, 2) for m in range(4)] + [(0, m, 3) for m in range(4)] +
          [(1, m, 2) for m in range(4)] + [(1, 0, 3), (1, 1, 3)] +
          [(4, m, 4) for m in range(4)])
CHAIN_GROUPS = [[0, 1, 2, 3], [4, 5, 6, 7], [8, 9, 10, 11],
                [14, 15, 16, 17], [12, 13]]
NCHAIN = len(CHAINS)


def _dma_order():
    """Input chunk order = strict consumption order: the first group's
    slots chunk-interleaved, then each later group's new slots in group
    order (warm MMs consume exactly at delivery rate, so any chunk
    loaded ahead of its use steals bandwidth from the active group)."""
    first = []
    for ci in CHAIN_GROUPS[0]:
        ls, _, rs = CHAINS[ci]
        for s in (ls, rs):
            if s not in first:
                first.append(s)
    rest = []
    for grp in CHAIN_GROUPS[1:]:
        for ci in grp:
            ls, _, rs = CHAINS[ci]
            for s in (ls, rs):
                if s not in first and s not in rest:
                    rest.append(s)
    order = []
    for c in range(NCHUNK):
        for s in first:
            order.append((s, c))
    for s in rest:
        for c in range(NCHUNK):
            order.append((s, c))
    return order


def _build_gram_kernel():
    import concourse.bacc as bacc
    import concourse.tile as tile
    from concourse import mybir

    nc = bacc.Bacc(None, target_bir_lowering=False,
                   enable_partition_id=False)

    f32 = mybir.dt.float32
    bf16 = mybir.dt.bfloat16
    fp8 = mybir.dt.float8e4

    grps = nc.declare_dram_parameter("grps", [NSLOT, 128, KT, B], fp8,
                                     isOutput=False)
    out = nc.declare_dram_parameter("out", [NCHAIN, 128, B], bf16,
                                    isOutput=True)

    with tile.TileContext(nc) as tc:
        with (
            tc.tile_pool(name="grp_p", bufs=1) as grp_pool,
            tc.tile_pool(name="psum", bufs=8, space="PSUM") as psum_pool,
            tc.tile_pool(name="outp", bufs=6) as out_pool,
        ):
            gch = [[grp_pool.tile([128, NCHUNK, B], fp8, name=f"g{s}_{c}")
                    for c in range(NCHUNK)] for s in range(NSLOT)]
            dmy = grp_pool.tile([128, 2, B], fp8, name="dmy")

            # First two critical chunks go out on the two parallel HWDGE
            # rings (SP + ACT) so their ~2us completion receipts overlap;
            # the rest stream on sync.
            order = _dma_order()
            (s0, c0), (s1, c1) = order[0], order[1]
            nc.sync.dma_start(gch[s0][c0][:],
                              grps[s0, :, c0 * NCHUNK:(c0 + 1) * NCHUNK, :])
            nc.scalar.dma_start(gch[s1][c1][:],
                                grps[s1, :, c1 * NCHUNK:(c1 + 1) * NCHUNK, :])
            for s, c in order[2:]:
                k0 = c * NCHUNK
                nc.sync.dma_start(gch[s][c][:], grps[s, :, k0:k0 + NCHUNK, :])

            # PE warm-up while the first chunks' ~2.5us HBM receipt is in
            # flight: 7 cold dummy matmuls (~8.2-11.2us) hold the HAM
            # activity window so the real chains start at full clock.
            nc.gpsimd.memset(dmy[:], 0.0)
            for i in range(7):
                wp = psum_pool.tile([128, B], f32, name="ps")
                nc.tensor.matmul(
                    wp[:], dmy[:, :, 0:128], dmy[:],
                    start=True, stop=True,
                    perf_mode=mybir.MatmulPerfMode.DoubleRow,
                )

            for grp in CHAIN_GROUPS:
                pss = [psum_pool.tile([128, B], f32, name="ps") for _ in grp]
                for t in range(TT):
                    ct = t // 2
                    o = 2 * (t % 2)
                    for j, ci in enumerate(grp):
                        ls, m, rs = CHAINS[ci]
                        nc.tensor.matmul(
                            pss[j][:],
                            gch[ls][ct][:, o:o + 2, m * 128:(m + 1) * 128],
                            gch[rs][ct][:, o:o + 2, :],
                            start=(t == 0),
                            stop=(t == TT - 1),
                            perf_mode=mybir.MatmulPerfMode.DoubleRow,
                        )
                for j, ci in enumerate(grp):
                    ot = out_pool.tile([128, B], bf16, name="ot")
                    # PSUM->SBUF casts alternate DVE/ACT (parallel PSUM
                    # ports); each chain's output DMA rides the other
                    # HWDGE ring than its cast engine so the tail
                    # parallelizes.
                    if ci % 2 == 0:
                        nc.vector.tensor_copy(ot[:], pss[j][:])
                        nc.scalar.dma_start(out[ci], ot[:])
                    else:
                        nc.scalar.copy(ot[:], pss[j][:])
                        nc.sync.dma_start(out[ci], ot[:])

    nc.compile()
    return nc


_NC_CACHE = None


def _pack_slot(eT8: np.ndarray, pair) -> np.ndarray:
    """eT8 [D, N] fp8 -> [128, KT, B] packed slot of two half-groups."""
    h0, h1 = pair
    blk = np.concatenate(
        [eT8[:, h0 * HALF:(h0 + 1) * HALF], eT8[:, h1 * HALF:(h1 + 1) * HALF]],
        axis=1)                                      # [2048, 512]
    return np.ascontiguousarray(
        blk.reshape(KT, 128, B).transpose(1, 0, 2))  # [128, 16, 512]


def _run_gram(emb: np.ndarray) -> np.ndarray:
    """Run the 8-core symmetric Gram kernel; returns G = emb @ emb.T f32."""
    global _NC_CACHE, LAST_EXEC_NS
    from concourse.bass_utils import run_bass_kernel_spmd

    if _NC_CACHE is None:
        _NC_CACHE = _build_gram_kernel()
    nc = _NC_CACHE

    eT8 = np.ascontiguousarray(emb.T).astype(ml_dtypes.float8_e4m3)
    pack_cache = {}
    in_maps = []
    for core in range(N_CORES):
        slabs = []
        for pair in SLOTPACK[core]:
            if pair not in pack_cache:
                pack_cache[pair] = _pack_slot(eT8, pair)
            slabs.append(pack_cache[pair])
        in_maps.append({"grps": np.ascontiguousarray(np.stack(slabs, axis=0))})

    trace = bool(int(os.environ.get("KERNEL_TRACE", "0")))
    res = run_bass_kernel_spmd(
        nc, in_maps, core_ids=list(range(N_CORES)), trace=trace
    )
    if res.exec_time_ns is not None:
        LAST_EXEC_NS = res.exec_time_ns

    G = np.empty((N, N), dtype=np.float32)
    for core in range(N_CORES):
        o = np.asarray(res.results[core]["out"], dtype=np.float32)  # [NCHAIN,128,B]
        S = SLOTPACK[core]
        for ci, (ls, m, rs) in enumerate(CHAINS):
            r0 = S[ls][m // 2] * HALF + (m % 2) * 128
            strip = o[ci]                       # [128, 512]
            for half in range(2):
                c0 = S[rs][half] * HALF
                piece = strip[:, half * HALF:(half + 1) * HALF]  # [128, 256]
                G[r0:r0 + 128, c0:c0 + HALF] = piece
                G[c0:c0 + HALF, r0:r0 + 128] = piece.T
    return G


def _sample_js(counts: np.ndarray, us: list) -> np.ndarray:
    """Replicate the reference's f32 sampling math. counts [N] int, us 3x[N]
    f32 uniforms. Returns j ranks [N, 3] int64 (rank into the masked sort)."""
    out = []
    for t, r in enumerate(RATIOS):
        cnt = np.maximum(
            np.int32(1),
            np.floor(counts.astype(np.float32) * np.float32(r)).astype(np.int32),
        )
        j = np.minimum((us[t] * cnt.astype(np.float32)).astype(np.int32), cnt - 1)
        out.append(j.astype(np.int64))
    return np.stack(out, axis=1)


def kernel(embeddings: np.ndarray, labels: np.ndarray) -> np.ndarray:
    emb = np.ascontiguousarray(np.asarray(embeddings, dtype=np.float32))
    lab = np.asarray(labels).astype(np.int64)

    G = _run_gram(emb)

    # Selection keys: within row i, ordering by (sq_j - 2 G[i,j]) equals
    # ordering by distance.
    sq = np.einsum("ij,ij->i", emb, emb).astype(np.float32)

    # Uniforms must match jax.random with key 42 bit-exactly.
    import jax

    with jax.default_device(jax.devices("cpu")[0]):
        skey = jax.random.key(42)
        keys = jax.random.split(skey, 6)
        us = [np.asarray(jax.random.uniform(k, (N,))) for k in keys]

    class_size = np.bincount(lab, minlength=NUM_IDS)
    pos_count = class_size[lab] - 1
    neg_count = N - class_size[lab]
    valid = (pos_count > 0) & (neg_count > 0)

    pos_js = _sample_js(pos_count, us[0:3])  # [N, 3]
    neg_js = _sample_js(neg_count, us[3:6])  # [N, 3]

    # Per-class member lists
    order = np.argsort(lab, kind="stable")
    sorted_lab = lab[order]
    starts = np.searchsorted(sorted_lab, np.arange(NUM_IDS), side="left")
    ends = np.searchsorted(sorted_lab, np.arange(NUM_IDS), side="right")

    pos_idx = np.zeros((N, 3), dtype=np.int64)
    neg_idx = np.zeros((N, 3), dtype=np.int64)
    INF = np.float32(np.inf)

    for i in range(N):
        li = lab[i]
        members = order[starts[li]:ends[li]]
        key_row = sq - 2.0 * G[i]  # f32 [N]
        if valid[i]:
            pos_members = members[members != i]
            pk = key_row[pos_members]
            po = np.argsort(pk, kind="stable")
            pos_idx[i] = pos_members[po[pos_js[i]]]
        # negatives: mask out own class and self
        nk = key_row.copy()
        nk[members] = INF
        nk[i] = INF
        kth = np.unique(neg_js[i])
        part = np.argpartition(nk, kth)
        neg_idx[i] = part[neg_js[i]]

    a = emb[:, None, :]
    p = emb[pos_idx]
    ng = emb[neg_idx]
    d_ap = np.sqrt(np.sum((a - p + np.float32(EPS)) ** 2, axis=-1))
    d_an = np.sqrt(np.sum((a - ng + np.float32(EPS)) ** 2, axis=-1))
    tri = np.maximum(d_ap - d_an + np.float32(MARGIN), np.float32(0.0))
    w = valid[:, None].astype(np.float32)
    denom = max(3.0 * float(valid.sum()), 1.0)
    loss = np.float32(np.sum(tri * w) / denom)
    return np.array(loss, dtype=np.float32)
